# revision 36
# baseline (speedup 1.0000x reference)
"""DILATE loss (soft-DTW shape + temporal) on 8 Trainium2 NeuronCores.

Strategy: the 256 (batch x channel) series are sharded 32 per core. Each
core runs a BANDED min-plus DP (band half-width W around the diagonal;
gamma=0.01 makes softmin ~min and the soft alignment posterior razor
sharp, so the band is lossless to well under the tolerance):

  D[i,j]   = (t_i - o_j)^2               (banded, j in [i-W, i+W])
  M[i,j]   = D + min(M[i-1,j-1], M[i-1,j], M[i,j-1])       (forward)
  num[i,j] = D + min(num[i+1,j+1], num[i+1,j], num[i,j+1]) (suffix)
  E*Omega  = exp(-lam*(M + num - D + womg - M[N,N])), womg = -ln(Om)/lam
  loss     = 0.5*sum(M[N,N])/B + 0.5*sum(E*Omega)/(B*T*T)

Layout: banded rows are spread over all 128 SBUF partitions as
(series s, row-group g): partition s+32g holds rows 32g..32g+31. Bulk
passes (D build, epilogue) then cost 1/4 the free-size. The DP rows run
on the 32 partitions of their group; group boundaries are crossed with
tiny SBUF->SBUF DMA hops.

Engine schedule: the fwd chain starts on DVE while the suffix chain
starts on GPSIMD; the chains swap engines mid-flight (FA/BA splits) so
both engines stay saturated. The epilogue is chunked so it pipelines
into the DP tail; Exp with accumulate runs on ACT.
"""
import sys
if "/opt/trn_rl_repo" not in sys.path:
    sys.path.insert(0, "/opt/trn_rl_repo")
import numpy as np
from contextlib import ExitStack

import concourse.bass as bass
import concourse.bacc as bacc
import concourse.mybir as mybir
import concourse.tile as tile
from concourse.mybir import AluOpType, ActivationFunctionType

F32 = mybir.dt.float32
S = 32            # series per core
N = 128           # T
G = 4             # partition row-groups
LR = N // G       # rows per group (32)
W = 24            # band half-width
Wb = 2 * W + 1    # banded row width (49)
RS = Wb + 1       # row stride in M/num tiles (one guard col)
OW = LR + Wb - 1  # o_grouped width (80)
LAM = 100.0
BIG = 1e30
SENT = 1e15       # o padding sentinel -> D ~ 1e30 outside the valid square
N_CORES = 8

DCH = 8           # D-build chunk rows
ECH = 8           # epilogue chunk rows
SOFF = 6          # bwd chain stagger (slots) so chain hops never collide


def gap(t, p0, pn, off, dims):
    """AP on partitions [p0, p0+pn) of tile t, free offset off, free dims."""
    base = t[p0:p0 + pn, 0:1]
    return bass.AP(base.tensor, base.offset + off, [base.ap[0]] + dims)


def _build_kernel():
    nc = bacc.Bacc("TRN2", target_bir_lowering=False, debug=False)
    to_d = nc.dram_tensor("to", [G * S, LR + OW], F32, kind="ExternalInput")
    wom_d = nc.dram_tensor("wom", [G * S, Wb], F32, kind="ExternalInput")
    pw_d = nc.dram_tensor("pw", [G * S, 3 * G * S], F32, kind="ExternalInput")
    vals_d = nc.dram_tensor("vals", [S, 1], F32, kind="ExternalOutput")
    tl_d = nc.dram_tensor("tl", [G * S, 1], F32, kind="ExternalOutput")

    NP = G * S  # 128 partitions

    with tile.TileContext(nc) as tc, ExitStack() as ctx:
        pool = ctx.enter_context(tc.tile_pool(name="main", bufs=1))
        psp = ctx.enter_context(tc.psum_pool(name="ps", bufs=1))
        to_s = pool.tile([NP, LR + OW], F32, tag="to_s")
        wom = pool.tile([NP, Wb], F32, tag="wom")
        pw_s = pool.tile([NP, 3 * NP], F32, tag="pw_s")
        pf = psp.tile([NP, RS], F32, tag="pf")
        pb = psp.tile([NP, RS], F32, tag="pb")
        pv = psp.tile([NP, 1], F32, tag="pv")
        Dg = pool.tile([NP, LR * Wb], F32, tag="Dg")
        Mt = pool.tile([NP, (LR + 1) * RS], F32, tag="Mt")
        Nt = pool.tile([NP, (LR + 1) * RS], F32, tag="Nt")
        entF = pool.tile([NP, Wb], F32, tag="entF")
        entB = pool.tile([NP, Wb], F32, tag="entB")
        Xg = pool.tile([NP, LR * Wb], F32, tag="Xg")
        Yg = pool.tile([NP, LR * Wb], F32, tag="Yg")
        bias = pool.tile([NP, 1], F32, tag="bias")
        tlp = pool.tile([NP, LR // ECH], F32, tag="tlp")
        tls = pool.tile([NP, 1], F32, tag="tls")

        # ---- init: guards and virtual boundary rows -------------------
        # M right-guard col + num left-guard col (all slots, all parts)
        nc.vector.memset(gap(Mt, 0, NP, Wb, [[RS, LR + 1], [1, 1]]), BIG)
        nc.vector.memset(gap(Nt, 0, NP, 0, [[RS, LR + 1], [1, 1]]), BIG)
        # fwd virtual row -1 on group 0: BIG except k=W (the DP origin)
        nc.vector.memset(Mt[0:S, 0:Wb], BIG)
        nc.vector.memset(Mt[0:S, W:W + 1], 0.0)
        # bwd virtual row 128 on group 3 (slot LR): BIG except k=W
        nc.gpsimd.memset(Nt[(G - 1) * S:NP, LR * RS + 1:LR * RS + 1 + Wb], BIG)
        nc.gpsimd.memset(Nt[(G - 1) * S:NP, LR * RS + 1 + W:LR * RS + 2 + W], 0.0)
        # hop-source slots: defined values on all partitions (the PE hop
        # matmul streams every partition; stray NaNs would poison PSUM)
        nc.gpsimd.memset(Mt[:, LR * RS:(LR + 1) * RS], BIG)
        nc.gpsimd.memset(Nt[:, 0:RS], BIG)

        # ---- input DMAs ----------------------------------------------
        nc.sync.dma_start(to_s[:], to_d.ap())
        nc.sync.dma_start(wom[:], wom_d.ap())
        nc.sync.dma_start(pw_s[:], pw_d.ap())

        # ---- D build: D = (t_bcast - o_sliding)^2 --------------------
        # first chunk fully on DVE (both chain heads unblock without any
        # cross-engine latency); the rest on Pool with Square on ACT.
        def d_chunk(c0, on_dve=False):
            dch = gap(Dg, 0, NP, c0 * Wb, [[Wb, DCH], [1, Wb]])
            t_ch = gap(to_s, 0, NP, c0, [[1, DCH], [0, Wb]])
            o_ch = gap(to_s, 0, NP, LR + c0, [[1, DCH], [1, Wb]])
            if on_dve:
                nc.vector.tensor_tensor(dch, t_ch, o_ch, AluOpType.subtract)
                nc.vector.tensor_tensor(dch, dch, dch, AluOpType.mult)
            else:
                nc.gpsimd.tensor_tensor(dch, t_ch, o_ch, AluOpType.subtract)
                nc.scalar.activation(dch, dch, ActivationFunctionType.Square)

        d_chunk(0, on_dve=True)
        d_chunk(LR - DCH)
        d_chunk(DCH)
        d_chunk(LR - 2 * DCH)

        # ---- DP rows: both chains interleaved 1:1 on DVE --------------
        def fwd_row(r):
            g, rho = r // LR, r % LR
            p0 = g * S
            if rho == 0 and g > 0:
                # land the PE-hopped boundary row from PSUM into slot 0
                nc.vector.tensor_copy(gap(Mt, p0, S, 0, [[1, RS]]),
                                      pf[p0:p0 + S, 0:RS])
            nc.vector.tensor_tensor(
                gap(entF, p0, S, 0, [[1, Wb]]),
                gap(Mt, p0, S, rho * RS, [[1, Wb]]),
                gap(Mt, p0, S, rho * RS + 1, [[1, Wb]]),
                AluOpType.min)
            nc.vector.tensor_tensor_scan(
                gap(Mt, p0, S, (rho + 1) * RS, [[1, Wb]]),
                gap(entF, p0, S, 0, [[1, Wb]]),
                gap(Dg, p0, S, rho * Wb, [[1, Wb]]),
                BIG, AluOpType.min, AluOpType.add)
            if rho == LR - 1 and g < G - 1:
                # hop row r to group g+1 via PE partition-shift matmul
                # (no DMA: avoids the 900ns DMA-completion semaphore)
                nc.tensor.matmul(pf[:, 0:RS], pw_s[:, 0:NP],
                             gap(Mt, 0, NP, LR * RS, [[1, RS]]),
                             start=True, stop=True)

        def bwd_row(i):
            g, rho = i // LR, i % LR
            p0 = g * S
            if rho == LR - 1 and g < G - 1:
                nc.vector.tensor_copy(gap(Nt, p0, S, LR * RS, [[1, RS]]),
                                      pb[p0:p0 + S, 0:RS])
            nc.vector.tensor_tensor(
                gap(entB, p0, S, 0, [[1, Wb]]),
                gap(Nt, p0, S, (rho + 1) * RS, [[1, Wb]]),
                gap(Nt, p0, S, (rho + 1) * RS + 1, [[1, Wb]]),
                AluOpType.min)
            nc.vector.tensor_tensor_scan(
                gap(Nt, p0, S, rho * RS + Wb, [[-1, Wb]]),
                gap(entB, p0, S, Wb - 1, [[-1, Wb]]),
                gap(Dg, p0, S, rho * Wb + Wb - 1, [[-1, Wb]]),
                BIG, AluOpType.min, AluOpType.add)
            if rho == 0 and g > 0:
                nc.tensor.matmul(pb[:, 0:RS], pw_s[:, NP:2 * NP],
                             gap(Nt, 0, NP, 0, [[1, RS]]),
                             start=True, stop=True)

        # interleave the two chains 1:1 (hides the ~240ns same-engine sem
        # latency); around each group-boundary hop, burst BN rows of the
        # opposite chain so the hop's DMA latency is fully covered.
        BN = 12
        fhop = {LR, 2 * LR, 3 * LR}          # fwd rows that wait on a hop
        bhop = {N - LR - 1, N - 2 * LR - 1, N - 3 * LR - 1}
        # rows that either trigger or consume a hop: a burst must not emit
        # one, or its DMA lands in the blocked row's DMA-sem window
        fstop = fhop | {r - 1 for r in fhop}
        bstop = bhop | {i + 1 for i in bhop}
        fi = 0
        bi = 0
        while fi < SOFF:
            fwd_row(fi)
            fi += 1
        while fi < N or bi < N:
            if bi < N:
                i = N - 1 - bi
                if i in bhop:
                    for _ in range(BN):
                        if fi < N and fi not in fstop:
                            fwd_row(fi)
                            fi += 1
                bwd_row(i)
                bi += 1
            if fi < N:
                r = fi
                if r in fhop:
                    for _ in range(BN):
                        if bi < N and (N - 1 - bi) not in bstop:
                            bwd_row(N - 1 - bi)
                            bi += 1
                fwd_row(r)
                fi += 1

        # Y = womg - D on Pool (independent, runs during the DP)
        nc.gpsimd.tensor_tensor(
            gap(Yg, 0, NP, 0, [[Wb, LR], [1, Wb]]),
            gap(wom, 0, NP, 0, [[0, LR], [1, Wb]]),
            gap(Dg, 0, NP, 0, [[Wb, LR], [1, Wb]]),
            AluOpType.subtract)

        # ---- bias = +lam * M[N,N] on every partition ------------------
        # PE replication matmul (cols 2NP:3NP of pw broadcast group 3's
        # vals to all groups) -> PSUM -> scale on DVE. No DMA is in this
        # dependency path (DMA-completion sems proved race-prone here).
        p3 = (G - 1) * S
        nc.tensor.matmul(pv[:, 0:1], pw_s[:, 2 * NP:3 * NP],
                         gap(Mt, 0, NP, LR * RS + W, [[1, 1]]),
                         start=True, stop=True)
        nc.vector.tensor_scalar(
            bias[:, 0:1], pv[:, 0:1], LAM, None, AluOpType.mult)
        nc.sync.dma_start(vals_d.ap(), gap(Mt, p3, S, LR * RS + W, [[1, 1]]))

        # ---- epilogue: X = M + num; X += Y; E*Om = Exp(-lam X + bias) -
        EXC = 2 * ECH  # exp chunk rows (fewer accum flushes)
        for ci in range(LR // ECH):
            c0 = ci * ECH
            xch = gap(Xg, 0, NP, c0 * Wb, [[Wb, ECH], [1, Wb]])
            nc.vector.tensor_tensor(
                xch,
                gap(Mt, 0, NP, (c0 + 1) * RS, [[RS, ECH], [1, Wb]]),
                gap(Nt, 0, NP, c0 * RS + 1, [[RS, ECH], [1, Wb]]),
                AluOpType.add)
            nc.vector.tensor_tensor(
                xch, xch, gap(Yg, 0, NP, c0 * Wb, [[Wb, ECH], [1, Wb]]),
                AluOpType.add)
            if ci % 2 == 1:
                e0 = (ci - 1) * ECH
                nc.scalar.activation(
                    gap(Yg, 0, NP, e0 * Wb, [[Wb, EXC], [1, Wb]]),
                    gap(Xg, 0, NP, e0 * Wb, [[Wb, EXC], [1, Wb]]),
                    ActivationFunctionType.Exp,
                    bias=bias[:, 0:1], scale=-LAM,
                    accum_out=tlp[:, ci // 2:ci // 2 + 1])
        nc.vector.tensor_reduce(tls[:], tlp[:, 0:LR // EXC],
                                mybir.AxisListType.X, AluOpType.add)
        nc.sync.dma_start(tl_d.ap(), tls[:])

    nc.compile()
    return nc


_NC_CACHE = None


def _get_nc():
    global _NC_CACHE
    if _NC_CACHE is None:
        _NC_CACHE = _build_kernel()
    return _NC_CACHE


def _host_inputs(outputs, targets):
    """Full inputs -> per-core grouped/banded host arrays, concatenated."""
    outputs = np.asarray(outputs, np.float32)
    targets = np.asarray(targets, np.float32)
    B, T, C = outputs.shape
    t = np.ascontiguousarray(
        np.transpose(targets, (0, 2, 1)).reshape(B * C, T))
    o = np.ascontiguousarray(
        np.transpose(outputs, (0, 2, 1)).reshape(B * C, T))
    # merged input: cols [0:LR] grouped t, cols [LR:LR+OW] padded grouped o
    to = np.empty((N_CORES, G, S, LR + OW), np.float32)
    to[:, :, :, :LR] = (t.reshape(N_CORES, S, G, LR)
                        .transpose(0, 2, 1, 3))
    opad = np.full((B * C, T + 2 * W), SENT, np.float32)
    opad[:, W:W + T] = o
    opad_c = opad.reshape(N_CORES, S, T + 2 * W)
    for g in range(G):
        to[:, g, :, LR:] = opad_c[:, :, g * LR:g * LR + OW]
    return np.ascontiguousarray(to.reshape(N_CORES * G * S, LR + OW))


def _womg():
    k = np.arange(Wb, dtype=np.float64)
    om = (W - k) ** 2
    w = np.where(om == 0.0, BIG, -np.log(np.maximum(om, 1e-30)) / LAM)
    w = np.broadcast_to(w.astype(np.float32), (N_CORES * G * S, Wb))
    return np.ascontiguousarray(w)


def _pshift():
    """PE weights: cols 0:128 shift +32 (fwd hops), cols 128:256 shift
    -32 (bwd hops), cols 256:384 broadcast group 3 to all groups."""
    NP = G * S
    k = np.arange(NP)[:, None]
    m = np.arange(NP)[None, :]
    wf = (k == m - S).astype(np.float32)
    wb = (k == m + S).astype(np.float32)
    w3 = (k == (G - 1) * S + (m % S)).astype(np.float32)
    pw = np.concatenate([wf, wb, w3], axis=1)
    return np.ascontiguousarray(np.tile(pw, (N_CORES, 1)))


_EXEC_CACHE = None


def _get_exec():
    """Build the sharded jitted executable once (mirrors bass2jax's
    run_bass_via_pjrt multi-core path)."""
    global _EXEC_CACHE
    if _EXEC_CACHE is not None:
        return _EXEC_CACHE
    import jax
    import concourse.mybir as _mybir
    from jax.sharding import Mesh, PartitionSpec, NamedSharding
    from jax.experimental.shard_map import shard_map
    from concourse.bass2jax import (
        _bass_exec_p, install_neuronx_cc_hook, partition_id_tensor)

    nc = _get_nc()
    install_neuronx_cc_hook()
    partition_name = nc.partition_id_tensor.name if nc.partition_id_tensor else None
    in_names, out_names, out_avals, zero_outs = [], [], [], []
    for alloc in nc.m.functions[0].allocations:
        if not isinstance(alloc, _mybir.MemoryLocationSet):
            continue
        name = alloc.memorylocations[0].name
        if alloc.kind == "ExternalInput":
            if name != partition_name:
                in_names.append(name)
        elif alloc.kind == "ExternalOutput":
            shape = tuple(alloc.tensor_shape)
            dtype = _mybir.dt.np(alloc.dtype)
            out_names.append(name)
            out_avals.append(jax.core.ShapedArray(shape, dtype))
            zero_outs.append(np.zeros(shape, dtype))
    n_params = len(in_names)
    all_in_names = list(in_names) + list(out_names)
    if partition_name is not None:
        all_in_names.append(partition_name)
    donate = tuple(range(n_params, n_params + len(out_names)))

    def _body(*args):
        operands = list(args)
        if partition_name is not None:
            operands.append(partition_id_tensor())
        return tuple(_bass_exec_p.bind(
            *operands,
            out_avals=tuple(out_avals),
            in_names=tuple(all_in_names),
            out_names=tuple(out_names),
            lowering_input_output_aliases=(),
            sim_require_finite=True,
            sim_require_nnan=True,
            nc=nc,
        ))

    devices = jax.devices()[:N_CORES]
    mesh = Mesh(np.asarray(devices), ("core",))
    in_specs = (PartitionSpec("core"),) * (n_params + len(out_names))
    out_specs = (PartitionSpec("core"),) * len(out_names)
    sharded = jax.jit(
        shard_map(_body, mesh=mesh, in_specs=in_specs, out_specs=out_specs,
                  check_rep=False),
        donate_argnums=donate, keep_unused=True)
    shard = NamedSharding(mesh, PartitionSpec("core"))
    wom_dev = jax.device_put(_womg(), shard)
    pw_dev = jax.device_put(_pshift(), shard)
    _EXEC_CACHE = (sharded, in_names, out_names, zero_outs, wom_dev, pw_dev)
    return _EXEC_CACHE


def kernel(outputs, targets):
    """outputs, targets: [64, 128, 4] float32 -> scalar float32 loss."""
    sharded, in_names, out_names, zero_outs, wom_dev, pw_dev = _get_exec()
    B, T, C = np.asarray(outputs).shape
    to = _host_inputs(outputs, targets)
    by_name = {"to": to, "wom": wom_dev, "pw": pw_dev}
    concat_in = [by_name[name] for name in in_names]
    concat_zeros = [
        np.zeros((N_CORES * z.shape[0], *z.shape[1:]), z.dtype)
        for z in zero_outs
    ]
    out_arrs = sharded(*concat_in, *concat_zeros)
    outs = {name: np.asarray(out_arrs[i]) for i, name in enumerate(out_names)}
    vals = outs["vals"][:, 0]
    tl = outs["tl"][:, 0]
    loss = 0.5 * (vals.sum(dtype=np.float64) / B) + \
           0.5 * (tl.sum(dtype=np.float64) / (B * T * T))
    return np.float32(loss)


# revision 37
# speedup vs baseline: 1.0804x; 1.0804x over previous
"""DILATE loss (soft-DTW shape + temporal) on 8 Trainium2 NeuronCores.

Strategy: the 256 (batch x channel) series are sharded 32 per core. Each
core runs a BANDED min-plus DP (band half-width W around the diagonal;
gamma=0.01 makes softmin ~min and the soft alignment posterior razor
sharp, so the band is lossless to well under the tolerance):

  D[i,j]   = (t_i - o_j)^2               (banded, j in [i-W, i+W])
  M[i,j]   = D + min(M[i-1,j-1], M[i-1,j], M[i,j-1])       (forward)
  num[i,j] = D + min(num[i+1,j+1], num[i+1,j], num[i,j+1]) (suffix)
  E*Omega  = exp(-lam*(M + num - D + womg - M[N,N])), womg = -ln(Om)/lam
  loss     = 0.5*sum(M[N,N])/B + 0.5*sum(E*Omega)/(B*T*T)

Layout: banded rows are spread over all 128 SBUF partitions as
(series s, row-group g): partition s+32g holds rows 32g..32g+31. Bulk
passes (D build, epilogue) then cost 1/4 the free-size. The DP rows run
on the 32 partitions of their group; group boundaries are crossed with
tiny SBUF->SBUF DMA hops.

Engine schedule: the fwd chain starts on DVE while the suffix chain
starts on GPSIMD; the chains swap engines mid-flight (FA/BA splits) so
both engines stay saturated. The epilogue is chunked so it pipelines
into the DP tail; Exp with accumulate runs on ACT.
"""
import sys
if "/opt/trn_rl_repo" not in sys.path:
    sys.path.insert(0, "/opt/trn_rl_repo")
import numpy as np
from contextlib import ExitStack

import concourse.bass as bass
import concourse.bacc as bacc
import concourse.mybir as mybir
import concourse.tile as tile
from concourse.mybir import AluOpType, ActivationFunctionType

F32 = mybir.dt.float32
S = 32            # series per core
N = 128           # T
G = 4             # partition row-groups
LR = N // G       # rows per group (32)
W = 20            # band half-width
Wb = 2 * W + 1    # banded row width (49)
RS = Wb + 1       # row stride in M/num tiles (one guard col)
OW = LR + Wb - 1  # o_grouped width (80)
LAM = 100.0
BIG = 1e30
SENT = 1e15       # o padding sentinel -> D ~ 1e30 outside the valid square
N_CORES = 8

DCH = 8           # D-build chunk rows
ECH = 8           # epilogue chunk rows
SOFF = 6          # bwd chain stagger (slots) so chain hops never collide


def gap(t, p0, pn, off, dims):
    """AP on partitions [p0, p0+pn) of tile t, free offset off, free dims."""
    base = t[p0:p0 + pn, 0:1]
    return bass.AP(base.tensor, base.offset + off, [base.ap[0]] + dims)


def _build_kernel():
    nc = bacc.Bacc("TRN2", target_bir_lowering=False, debug=False)
    to_d = nc.dram_tensor("to", [G * S, LR + OW], F32, kind="ExternalInput")
    wom_d = nc.dram_tensor("wom", [G * S, Wb], F32, kind="ExternalInput")
    pw_d = nc.dram_tensor("pw", [G * S, 3 * G * S], F32, kind="ExternalInput")
    vals_d = nc.dram_tensor("vals", [S, 1], F32, kind="ExternalOutput")
    tl_d = nc.dram_tensor("tl", [G * S, 1], F32, kind="ExternalOutput")

    NP = G * S  # 128 partitions

    with tile.TileContext(nc) as tc, ExitStack() as ctx:
        pool = ctx.enter_context(tc.tile_pool(name="main", bufs=1))
        psp = ctx.enter_context(tc.psum_pool(name="ps", bufs=1))
        to_s = pool.tile([NP, LR + OW], F32, tag="to_s")
        wom = pool.tile([NP, Wb], F32, tag="wom")
        pw_s = pool.tile([NP, 3 * NP], F32, tag="pw_s")
        pf = psp.tile([NP, RS], F32, tag="pf")
        pb = psp.tile([NP, RS], F32, tag="pb")
        pv = psp.tile([NP, 1], F32, tag="pv")
        Dg = pool.tile([NP, LR * Wb], F32, tag="Dg")
        Mt = pool.tile([NP, (LR + 1) * RS], F32, tag="Mt")
        Nt = pool.tile([NP, (LR + 1) * RS], F32, tag="Nt")
        entF = pool.tile([NP, Wb], F32, tag="entF")
        entB = pool.tile([NP, Wb], F32, tag="entB")
        Xg = pool.tile([NP, LR * Wb], F32, tag="Xg")
        Yg = pool.tile([NP, LR * Wb], F32, tag="Yg")
        bias = pool.tile([NP, 1], F32, tag="bias")
        tlp = pool.tile([NP, LR // ECH], F32, tag="tlp")
        tls = pool.tile([NP, 1], F32, tag="tls")

        # ---- init: guards and virtual boundary rows -------------------
        # M right-guard col + num left-guard col (all slots, all parts)
        nc.vector.memset(gap(Mt, 0, NP, Wb, [[RS, LR + 1], [1, 1]]), BIG)
        nc.vector.memset(gap(Nt, 0, NP, 0, [[RS, LR + 1], [1, 1]]), BIG)
        # fwd virtual row -1 on group 0: BIG except k=W (the DP origin)
        nc.vector.memset(Mt[0:S, 0:Wb], BIG)
        nc.vector.memset(Mt[0:S, W:W + 1], 0.0)
        # bwd virtual row 128 on group 3 (slot LR): BIG except k=W
        nc.gpsimd.memset(Nt[(G - 1) * S:NP, LR * RS + 1:LR * RS + 1 + Wb], BIG)
        nc.gpsimd.memset(Nt[(G - 1) * S:NP, LR * RS + 1 + W:LR * RS + 2 + W], 0.0)
        # hop-source slots: defined values on all partitions (the PE hop
        # matmul streams every partition; stray NaNs would poison PSUM)
        nc.gpsimd.memset(Mt[:, LR * RS:(LR + 1) * RS], BIG)
        nc.gpsimd.memset(Nt[:, 0:RS], BIG)

        # ---- input DMAs ----------------------------------------------
        nc.sync.dma_start(to_s[:], to_d.ap())
        nc.sync.dma_start(wom[:], wom_d.ap())
        nc.sync.dma_start(pw_s[:], pw_d.ap())

        # ---- D build: D = (t_bcast - o_sliding)^2 --------------------
        # first chunk fully on DVE (both chain heads unblock without any
        # cross-engine latency); the rest on Pool with Square on ACT.
        def d_chunk(c0, on_dve=False):
            dch = gap(Dg, 0, NP, c0 * Wb, [[Wb, DCH], [1, Wb]])
            t_ch = gap(to_s, 0, NP, c0, [[1, DCH], [0, Wb]])
            o_ch = gap(to_s, 0, NP, LR + c0, [[1, DCH], [1, Wb]])
            if on_dve:
                nc.vector.tensor_tensor(dch, t_ch, o_ch, AluOpType.subtract)
                nc.vector.tensor_tensor(dch, dch, dch, AluOpType.mult)
            else:
                nc.gpsimd.tensor_tensor(dch, t_ch, o_ch, AluOpType.subtract)
                nc.scalar.activation(dch, dch, ActivationFunctionType.Square)

        d_chunk(0, on_dve=True)
        d_chunk(LR - DCH)
        d_chunk(DCH)
        d_chunk(LR - 2 * DCH)

        # ---- DP rows: both chains interleaved 1:1 on DVE --------------
        def fwd_row(r):
            g, rho = r // LR, r % LR
            p0 = g * S
            if rho == 0 and g > 0:
                # land the PE-hopped boundary row from PSUM into slot 0
                nc.vector.tensor_copy(gap(Mt, p0, S, 0, [[1, RS]]),
                                      pf[p0:p0 + S, 0:RS])
            nc.vector.tensor_tensor(
                gap(entF, p0, S, 0, [[1, Wb]]),
                gap(Mt, p0, S, rho * RS, [[1, Wb]]),
                gap(Mt, p0, S, rho * RS + 1, [[1, Wb]]),
                AluOpType.min)
            nc.vector.tensor_tensor_scan(
                gap(Mt, p0, S, (rho + 1) * RS, [[1, Wb]]),
                gap(entF, p0, S, 0, [[1, Wb]]),
                gap(Dg, p0, S, rho * Wb, [[1, Wb]]),
                BIG, AluOpType.min, AluOpType.add)
            if rho == LR - 1 and g < G - 1:
                # hop row r to group g+1 via PE partition-shift matmul
                # (no DMA: avoids the 900ns DMA-completion semaphore)
                nc.tensor.matmul(pf[:, 0:RS], pw_s[:, 0:NP],
                             gap(Mt, 0, NP, LR * RS, [[1, RS]]),
                             start=True, stop=True)

        def bwd_row(i):
            g, rho = i // LR, i % LR
            p0 = g * S
            if rho == LR - 1 and g < G - 1:
                nc.vector.tensor_copy(gap(Nt, p0, S, LR * RS, [[1, RS]]),
                                      pb[p0:p0 + S, 0:RS])
            nc.vector.tensor_tensor(
                gap(entB, p0, S, 0, [[1, Wb]]),
                gap(Nt, p0, S, (rho + 1) * RS, [[1, Wb]]),
                gap(Nt, p0, S, (rho + 1) * RS + 1, [[1, Wb]]),
                AluOpType.min)
            nc.vector.tensor_tensor_scan(
                gap(Nt, p0, S, rho * RS + Wb, [[-1, Wb]]),
                gap(entB, p0, S, Wb - 1, [[-1, Wb]]),
                gap(Dg, p0, S, rho * Wb + Wb - 1, [[-1, Wb]]),
                BIG, AluOpType.min, AluOpType.add)
            if rho == 0 and g > 0:
                nc.tensor.matmul(pb[:, 0:RS], pw_s[:, NP:2 * NP],
                             gap(Nt, 0, NP, 0, [[1, RS]]),
                             start=True, stop=True)

        # interleave the two chains 1:1 (hides the ~240ns same-engine sem
        # latency); around each group-boundary hop, burst BN rows of the
        # opposite chain so the hop's DMA latency is fully covered.
        BN = 12
        fhop = {LR, 2 * LR, 3 * LR}          # fwd rows that wait on a hop
        bhop = {N - LR - 1, N - 2 * LR - 1, N - 3 * LR - 1}
        # rows that either trigger or consume a hop: a burst must not emit
        # one, or its DMA lands in the blocked row's DMA-sem window
        fstop = fhop | {r - 1 for r in fhop}
        bstop = bhop | {i + 1 for i in bhop}
        fi = 0
        bi = 0
        while fi < SOFF:
            fwd_row(fi)
            fi += 1
        while fi < N or bi < N:
            if bi < N:
                i = N - 1 - bi
                if i in bhop:
                    for _ in range(BN):
                        if fi < N and fi not in fstop:
                            fwd_row(fi)
                            fi += 1
                bwd_row(i)
                bi += 1
            if fi < N:
                r = fi
                if r in fhop:
                    for _ in range(BN):
                        if bi < N and (N - 1 - bi) not in bstop:
                            bwd_row(N - 1 - bi)
                            bi += 1
                fwd_row(r)
                fi += 1

        # Y = womg - D on Pool (independent, runs during the DP)
        nc.gpsimd.tensor_tensor(
            gap(Yg, 0, NP, 0, [[Wb, LR], [1, Wb]]),
            gap(wom, 0, NP, 0, [[0, LR], [1, Wb]]),
            gap(Dg, 0, NP, 0, [[Wb, LR], [1, Wb]]),
            AluOpType.subtract)

        # ---- bias = +lam * M[N,N] on every partition ------------------
        # PE replication matmul (cols 2NP:3NP of pw broadcast group 3's
        # vals to all groups) -> PSUM -> scale on DVE. No DMA is in this
        # dependency path (DMA-completion sems proved race-prone here).
        p3 = (G - 1) * S
        nc.tensor.matmul(pv[:, 0:1], pw_s[:, 2 * NP:3 * NP],
                         gap(Mt, 0, NP, LR * RS + W, [[1, 1]]),
                         start=True, stop=True)
        nc.vector.tensor_scalar(
            bias[:, 0:1], pv[:, 0:1], LAM, None, AluOpType.mult)
        nc.sync.dma_start(vals_d.ap(), gap(Mt, p3, S, LR * RS + W, [[1, 1]]))

        # ---- epilogue: X = M + num; X += Y; E*Om = Exp(-lam X + bias) -
        EXC = 2 * ECH  # exp chunk rows (fewer accum flushes)
        for ci in range(LR // ECH):
            c0 = ci * ECH
            xch = gap(Xg, 0, NP, c0 * Wb, [[Wb, ECH], [1, Wb]])
            nc.vector.tensor_tensor(
                xch,
                gap(Mt, 0, NP, (c0 + 1) * RS, [[RS, ECH], [1, Wb]]),
                gap(Nt, 0, NP, c0 * RS + 1, [[RS, ECH], [1, Wb]]),
                AluOpType.add)
            nc.vector.tensor_tensor(
                xch, xch, gap(Yg, 0, NP, c0 * Wb, [[Wb, ECH], [1, Wb]]),
                AluOpType.add)
            if ci % 2 == 1:
                e0 = (ci - 1) * ECH
                nc.scalar.activation(
                    gap(Yg, 0, NP, e0 * Wb, [[Wb, EXC], [1, Wb]]),
                    gap(Xg, 0, NP, e0 * Wb, [[Wb, EXC], [1, Wb]]),
                    ActivationFunctionType.Exp,
                    bias=bias[:, 0:1], scale=-LAM,
                    accum_out=tlp[:, ci // 2:ci // 2 + 1])
        nc.vector.tensor_reduce(tls[:], tlp[:, 0:LR // EXC],
                                mybir.AxisListType.X, AluOpType.add)
        nc.sync.dma_start(tl_d.ap(), tls[:])

    nc.compile()
    return nc


_NC_CACHE = None


def _get_nc():
    global _NC_CACHE
    if _NC_CACHE is None:
        _NC_CACHE = _build_kernel()
    return _NC_CACHE


def _host_inputs(outputs, targets):
    """Full inputs -> per-core grouped/banded host arrays, concatenated."""
    outputs = np.asarray(outputs, np.float32)
    targets = np.asarray(targets, np.float32)
    B, T, C = outputs.shape
    t = np.ascontiguousarray(
        np.transpose(targets, (0, 2, 1)).reshape(B * C, T))
    o = np.ascontiguousarray(
        np.transpose(outputs, (0, 2, 1)).reshape(B * C, T))
    # merged input: cols [0:LR] grouped t, cols [LR:LR+OW] padded grouped o
    to = np.empty((N_CORES, G, S, LR + OW), np.float32)
    to[:, :, :, :LR] = (t.reshape(N_CORES, S, G, LR)
                        .transpose(0, 2, 1, 3))
    opad = np.full((B * C, T + 2 * W), SENT, np.float32)
    opad[:, W:W + T] = o
    opad_c = opad.reshape(N_CORES, S, T + 2 * W)
    for g in range(G):
        to[:, g, :, LR:] = opad_c[:, :, g * LR:g * LR + OW]
    return np.ascontiguousarray(to.reshape(N_CORES * G * S, LR + OW))


def _womg():
    k = np.arange(Wb, dtype=np.float64)
    om = (W - k) ** 2
    w = np.where(om == 0.0, BIG, -np.log(np.maximum(om, 1e-30)) / LAM)
    w = np.broadcast_to(w.astype(np.float32), (N_CORES * G * S, Wb))
    return np.ascontiguousarray(w)


def _pshift():
    """PE weights: cols 0:128 shift +32 (fwd hops), cols 128:256 shift
    -32 (bwd hops), cols 256:384 broadcast group 3 to all groups."""
    NP = G * S
    k = np.arange(NP)[:, None]
    m = np.arange(NP)[None, :]
    wf = (k == m - S).astype(np.float32)
    wb = (k == m + S).astype(np.float32)
    w3 = (k == (G - 1) * S + (m % S)).astype(np.float32)
    pw = np.concatenate([wf, wb, w3], axis=1)
    return np.ascontiguousarray(np.tile(pw, (N_CORES, 1)))


_EXEC_CACHE = None


def _get_exec():
    """Build the sharded jitted executable once (mirrors bass2jax's
    run_bass_via_pjrt multi-core path)."""
    global _EXEC_CACHE
    if _EXEC_CACHE is not None:
        return _EXEC_CACHE
    import jax
    import concourse.mybir as _mybir
    from jax.sharding import Mesh, PartitionSpec, NamedSharding
    from jax.experimental.shard_map import shard_map
    from concourse.bass2jax import (
        _bass_exec_p, install_neuronx_cc_hook, partition_id_tensor)

    nc = _get_nc()
    install_neuronx_cc_hook()
    partition_name = nc.partition_id_tensor.name if nc.partition_id_tensor else None
    in_names, out_names, out_avals, zero_outs = [], [], [], []
    for alloc in nc.m.functions[0].allocations:
        if not isinstance(alloc, _mybir.MemoryLocationSet):
            continue
        name = alloc.memorylocations[0].name
        if alloc.kind == "ExternalInput":
            if name != partition_name:
                in_names.append(name)
        elif alloc.kind == "ExternalOutput":
            shape = tuple(alloc.tensor_shape)
            dtype = _mybir.dt.np(alloc.dtype)
            out_names.append(name)
            out_avals.append(jax.core.ShapedArray(shape, dtype))
            zero_outs.append(np.zeros(shape, dtype))
    n_params = len(in_names)
    all_in_names = list(in_names) + list(out_names)
    if partition_name is not None:
        all_in_names.append(partition_name)
    donate = tuple(range(n_params, n_params + len(out_names)))

    def _body(*args):
        operands = list(args)
        if partition_name is not None:
            operands.append(partition_id_tensor())
        return tuple(_bass_exec_p.bind(
            *operands,
            out_avals=tuple(out_avals),
            in_names=tuple(all_in_names),
            out_names=tuple(out_names),
            lowering_input_output_aliases=(),
            sim_require_finite=True,
            sim_require_nnan=True,
            nc=nc,
        ))

    devices = jax.devices()[:N_CORES]
    mesh = Mesh(np.asarray(devices), ("core",))
    in_specs = (PartitionSpec("core"),) * (n_params + len(out_names))
    out_specs = (PartitionSpec("core"),) * len(out_names)
    sharded = jax.jit(
        shard_map(_body, mesh=mesh, in_specs=in_specs, out_specs=out_specs,
                  check_rep=False),
        donate_argnums=donate, keep_unused=True)
    shard = NamedSharding(mesh, PartitionSpec("core"))
    wom_dev = jax.device_put(_womg(), shard)
    pw_dev = jax.device_put(_pshift(), shard)
    _EXEC_CACHE = (sharded, in_names, out_names, zero_outs, wom_dev, pw_dev)
    return _EXEC_CACHE


def kernel(outputs, targets):
    """outputs, targets: [64, 128, 4] float32 -> scalar float32 loss."""
    sharded, in_names, out_names, zero_outs, wom_dev, pw_dev = _get_exec()
    B, T, C = np.asarray(outputs).shape
    to = _host_inputs(outputs, targets)
    by_name = {"to": to, "wom": wom_dev, "pw": pw_dev}
    concat_in = [by_name[name] for name in in_names]
    concat_zeros = [
        np.zeros((N_CORES * z.shape[0], *z.shape[1:]), z.dtype)
        for z in zero_outs
    ]
    out_arrs = sharded(*concat_in, *concat_zeros)
    outs = {name: np.asarray(out_arrs[i]) for i, name in enumerate(out_names)}
    vals = outs["vals"][:, 0]
    tl = outs["tl"][:, 0]
    loss = 0.5 * (vals.sum(dtype=np.float64) / B) + \
           0.5 * (tl.sum(dtype=np.float64) / (B * T * T))
    return np.float32(loss)
